# revision 8
# baseline (speedup 1.0000x reference)
"""Trainium2 Bass kernel for ChunkMessagePassing (gnn_message_passing).

Problem: B=2, N=4096, D=512, 3 rounds of causal windowed (W=8) message
passing. Per round:
    A = h @ w1_top + b1 ; Bv = h @ w1_bot          (first MLP layer, factored)
    S[i] = sum_{k=0..8, valid} gelu(A[i] + Bv[i-k])
    agg[i] = (S[i]/cnt[i]) @ w2 + b2               (sum commutes with linear)
    new_h = h + MLP_upd([h, agg]) ; h = LN(new_h)

Sharding: 8 cores = B(2) x N-quarters(4). Each core gets 1024 tokens plus a
24-token left halo (3 rounds x window 8), computed redundantly. Zero
cross-core communication. Cores at a sequence start get a zero pad instead
of a halo plus a data-driven edge fixup (invalid window taps excluded, window
count < 9) so all 8 cores run one SPMD program.

Layout: D on partitions (4 tiles of 128), tokens on the free axis.
Matmuls in fp32r (1 cyc/row, ~1e-4 rel err). Window stage in bf16 for DVE
2x mode; shifted reads stay 4B-aligned via an extra odd-shifted Bv copy.
LN stats via ones-matmul over partitions, broadcast back via K=1 matmul.
"""

import numpy as np
import ml_dtypes

import concourse.bacc as bacc
import concourse.mybir as mybir
from concourse.tile import TileContext
from concourse.bass_utils import run_bass_kernel_spmd

f32 = mybir.dt.float32
f32r = mybir.dt.float32r
bf16 = mybir.dt.bfloat16
AF = mybir.ActivationFunctionType
ALU = mybir.AluOpType

B, N, D = 2, 4096, 512
N_ROUNDS = 3
W = 8
W9 = W + 1
NCORES = 8
NLOC = N // 4            # tokens owned per core
HALO = N_ROUNDS * W      # 24
T = NLOC + HALO          # 1048 local tokens incl. halo
DT = 4                   # number of 128-partition d tiles
P = 128
MARG = 8                 # zero margin on the left of Bv buffers
CHUNKS = [(0, 352), (352, 352), (704, 344)]   # psum-stage chunks of T
HC = T // 2              # window-stage half chunk (524)
EPS = 1e-5


def build_nc():
    nc = bacc.Bacc("TRN2")

    # ---- DRAM I/O (per-core data supplied via in_maps) ----
    h_in = nc.dram_tensor("h_in", [DT, P, T], f32r, kind="ExternalInput")
    w1t_d = nc.dram_tensor("w1t", [DT, P, D], f32r, kind="ExternalInput")
    w1b_d = nc.dram_tensor("w1b", [DT, P, D], f32r, kind="ExternalInput")
    u1t_d = nc.dram_tensor("u1t", [DT, P, D], f32r, kind="ExternalInput")
    u1b_d = nc.dram_tensor("u1b", [DT, P, D], bf16, kind="ExternalInput")
    w2s_d = nc.dram_tensor("w2s", [DT, P, D], bf16, kind="ExternalInput")
    u2_d = nc.dram_tensor("u2", [DT, P, D], bf16, kind="ExternalInput")
    # biases packed (P, DT) column dt; ln gamma/beta too
    b1_d = nc.dram_tensor("b1", [P, DT], f32, kind="ExternalInput")
    b2_d = nc.dram_tensor("b2", [P, DT], f32, kind="ExternalInput")
    ub1_d = nc.dram_tensor("ub1", [P, DT], f32, kind="ExternalInput")
    ub2_d = nc.dram_tensor("ub2", [P, DT], f32, kind="ExternalInput")
    lng_d = nc.dram_tensor("lng", [P, DT], f32, kind="ExternalInput")
    lnb_d = nc.dram_tensor("lnb", [P, DT], f32, kind="ExternalInput")
    iden_d = nc.dram_tensor("iden", [P, P], f32r, kind="ExternalInput")
    # per-core edge constants
    ea_d = nc.dram_tensor("edge_a", [P, W], bf16, kind="ExternalInput")
    es_d = nc.dram_tensor("edge_s", [P, W], bf16, kind="ExternalInput")
    hm_d = nc.dram_tensor("hmask", [P, HALO], f32, kind="ExternalInput")
    out_d = nc.dram_tensor("out", [DT, P, NLOC], f32, kind="ExternalOutput")

    with nc.allow_low_precision("bf16/f32r compute validated against reference"), \
            TileContext(nc) as tc:
        with (
            tc.tile_pool(name="const", bufs=1) as cp,
            tc.tile_pool(name="acts", bufs=1) as ap,
            tc.tile_pool(name="wsc", bufs=2) as wp,
            tc.tile_pool(name="ps", bufs=5, space="PSUM") as ps,
            tc.tile_pool(name="psr", bufs=3, space="PSUM") as psr,
        ):
            # ---- constants into SBUF ----
            w1t = cp.tile([P, DT * D], f32r, tag="w1t")
            w1b = cp.tile([P, DT * D], f32r, tag="w1b")
            u1t = cp.tile([P, DT * D], f32r, tag="u1t")
            u1b = cp.tile([P, DT * D], bf16, tag="u1b")
            w2s = cp.tile([P, DT * D], bf16, tag="w2s")
            u2 = cp.tile([P, DT * D], bf16, tag="u2")
            iden = cp.tile([P, P], f32r, tag="iden")
            for t_sb, t_d in ((w1t, w1t_d), (w1b, w1b_d), (u1t, u1t_d),
                              (u1b, u1b_d), (w2s, w2s_d), (u2, u2_d)):
                for dt in range(DT):
                    nc.sync.dma_start(out=t_sb[:, dt * D:(dt + 1) * D], in_=t_d[dt])
            nc.sync.dma_start(out=iden[:], in_=iden_d[:])
            b1 = cp.tile([P, DT], f32, tag="b1")
            b2 = cp.tile([P, DT], f32, tag="b2")
            ub1 = cp.tile([P, DT], f32, tag="ub1")
            ub2 = cp.tile([P, DT], f32, tag="ub2")
            lng = cp.tile([P, DT], f32, tag="lng")
            lnb = cp.tile([P, DT], f32, tag="lnb")
            edge_a = cp.tile([P, W], bf16, tag="edge_a")
            edge_s = cp.tile([P, W], bf16, tag="edge_s")
            hmask = cp.tile([P, HALO], f32, tag="hmask")
            for t_sb, t_d in ((b1, b1_d), (b2, b2_d), (ub1, ub1_d), (ub2, ub2_d),
                              (lng, lng_d), (lnb, lnb_d), (edge_a, ea_d),
                              (edge_s, es_d), (hmask, hm_d)):
                nc.sync.dma_start(out=t_sb[:], in_=t_d[:])

            ones_col = cp.tile([P, 1], f32r, tag="ones_col")   # stats lhsT (K=128, M=1)
            ones_row = cp.tile([1, P], f32r, tag="ones_row")   # bcast lhsT (K=1, M=128)
            ones_f = cp.tile([P, 1], f32, tag="ones_f")
            nc.vector.memset(ones_f[:], 1.0)
            nc.vector.tensor_copy(ones_col[:], ones_f[:])
            nc.vector.tensor_copy(ones_row[:], ones_f[:1, :].to_broadcast([1, P]))
            # const APs for float biases in activation() (0.0 default, eps)
            czero = cp.tile([P, 1], f32, tag="czero")
            ceps = cp.tile([P, 1], f32, tag="ceps")
            nc.vector.memset(czero[:], 0.0)
            nc.vector.memset(ceps[:], EPS)
            nc.const_aps.aps[(f32, 0.0)] = czero[:]
            nc.const_aps.aps[(f32, EPS)] = ceps[:]

            # ---- activations (persistent, reused across rounds) ----
            h0 = ap.tile([P, DT * T], f32r, tag="h0")
            h1 = ap.tile([P, DT * T], f32r, tag="h1")
            A = ap.tile([P, DT * T], bf16, tag="A")
            BVW = MARG + T + 2        # small tail pad, keeps strides even
            Bv = ap.tile([P, DT * BVW], bf16, tag="Bv")
            Bvo = ap.tile([P, DT * BVW], bf16, tag="Bvo")
            S = ap.tile([P, DT * T], bf16, tag="S")
            agg = ap.tile([P, DT * T], bf16, tag="agg")
            x2 = ap.tile([P, DT * T], f32r, tag="x2")
            rowAB = ap.tile([1, 2 * T], f32r, tag="rowAB")   # [-mu*rstd | rstd]
            rowCD = ap.tile([1, 2 * T], f32, tag="rowCD")    # scratch
            ga8 = ap.tile([P, W], bf16, tag="ga8")
            # xn aliases x2: x2[*,c] is dead (stats consumed) before xn[*,c]
            # is written; Tile serializes the WAR. G aliases A likewise.
            xn = x2
            G = A

            # zero Bv margins once (per-dt row of MARG cols at block start)
            for dt in range(DT):
                nc.vector.memset(Bv[:, dt * BVW: dt * BVW + MARG], 0.0)
                nc.vector.memset(Bvo[:, dt * BVW: dt * BVW + MARG + 1], 0.0)

            # round-1 input: DMA straight into h0 (f32r dram -> f32r sbuf)
            for dt in range(DT):
                nc.sync.dma_start(out=h0[:, dt * T:(dt + 1) * T], in_=h_in[dt])

            def hsl(h, dt, c0, n):
                return h[:, dt * T + c0: dt * T + c0 + n]

            hbufs = [h0, h1]
            for r in range(N_ROUNDS):
                hin = hbufs[r % 2]
                hout = hbufs[(r + 1) % 2]

                # ---- stage 1: A = w1t.T@h (+b1 later via ACT bias), Bv = w1b.T@h
                for dt in range(DT):
                    for (c0, cn) in CHUNKS:
                        pa = ps.tile([P, 512], f32, tag="pmm")
                        for kt in range(DT):
                            nc.tensor.matmul(
                                pa[:, :cn],
                                w1t[:, kt * D + dt * P: kt * D + dt * P + P],
                                hsl(hin, kt, c0, cn),
                                start=(kt == 0), stop=(kt == DT - 1),
                            )
                        nc.scalar.activation(A[:, dt * T + c0: dt * T + c0 + cn],
                                             pa[:, :cn], AF.Copy)
                        pb = ps.tile([P, 512], f32, tag="pmm")
                        for kt in range(DT):
                            nc.tensor.matmul(
                                pb[:, :cn],
                                w1b[:, kt * D + dt * P: kt * D + dt * P + P],
                                hsl(hin, kt, c0, cn),
                                start=(kt == 0), stop=(kt == DT - 1),
                            )
                        base = dt * BVW + MARG + c0
                        nc.scalar.activation(Bv[:, base: base + cn], pb[:, :cn], AF.Copy)
                        nc.scalar.activation(Bvo[:, base + 1: base + 1 + cn], pb[:, :cn], AF.Copy)

                # ---- stage 2: windowed gelu-sum -> S
                for dt in range(DT):
                    for hc0 in (0, HC):
                        tmp = wp.tile([P, W9 * HC], bf16, tag="tmp")
                        g = wp.tile([P, W9 * HC], bf16, tag="g")
                        a_sl = A[:, dt * T + hc0: dt * T + hc0 + HC]
                        for k in range(W9):
                            # Bv[i-k]: even k from Bv (base MARG-k), odd k from
                            # Bvo (stored shifted +1; base MARG+1-k) - keeps
                            # 4B alignment for bf16 2x mode
                            if k % 2 == 0:
                                src = Bv[:, dt * BVW + MARG - k + hc0:
                                         dt * BVW + MARG - k + hc0 + HC]
                            else:
                                src = Bvo[:, dt * BVW + MARG + 1 - k + hc0:
                                          dt * BVW + MARG + 1 - k + hc0 + HC]
                            nc.vector.tensor_tensor(
                                tmp[:, k * HC:(k + 1) * HC], a_sl, src, ALU.add)
                        nc.scalar.activation(g[:], tmp[:], AF.Gelu,
                                             bias=b1[:, dt: dt + 1])
                        # tree-sum the 9 blocks (contiguous halves)
                        nc.vector.tensor_tensor(
                            tmp[:, 0: 4 * HC], g[:, 0: 4 * HC], g[:, 4 * HC: 8 * HC], ALU.add)
                        nc.vector.tensor_tensor(
                            tmp[:, 0: 2 * HC], tmp[:, 0: 2 * HC], tmp[:, 2 * HC: 4 * HC], ALU.add)
                        nc.vector.tensor_tensor(
                            tmp[:, 0: HC], tmp[:, 0: HC], tmp[:, HC: 2 * HC], ALU.add)
                        nc.vector.tensor_tensor(
                            S[:, dt * T + hc0: dt * T + hc0 + HC],
                            tmp[:, 0: HC], g[:, 8 * HC: 9 * HC], ALU.add)

                # ---- edge fixup (no-op on non-sequence-start cores) ----
                # S[i] := (S[i] - (W-i)*gelu(A[i]+b1)) * 9/(i+1) on local
                # columns HALO..HALO+8 (global tokens 0..7); edge_a/edge_s are
                # zeros/ones elsewhere by construction.
                for dt in range(DT):
                    sle = S[:, dt * T + HALO: dt * T + HALO + W]
                    nc.scalar.activation(ga8[:], A[:, dt * T + HALO: dt * T + HALO + W],
                                         AF.Gelu, bias=b1[:, dt: dt + 1])
                    nc.vector.tensor_tensor(ga8[:], ga8[:], edge_a[:], ALU.mult)
                    nc.vector.tensor_tensor(sle, sle, ga8[:], ALU.subtract)
                    nc.vector.tensor_tensor(sle, sle, edge_s[:], ALU.mult)

                # ---- stage 3: agg = S @ w2s + b2 (w2s pre-scaled by 1/9)
                for dt in range(DT):
                    for (c0, cn) in CHUNKS:
                        pg = ps.tile([P, 512], f32, tag="pmm")
                        for kt in range(DT):
                            nc.tensor.matmul(
                                pg[:, :cn],
                                w2s[:, kt * D + dt * P: kt * D + dt * P + P],
                                S[:, kt * T + c0: kt * T + c0 + cn],
                                start=(kt == 0), stop=(kt == DT - 1),
                            )
                        nc.scalar.activation(agg[:, dt * T + c0: dt * T + c0 + cn],
                                             pg[:, :cn], AF.Identity,
                                             bias=b2[:, dt: dt + 1])

                # ---- stage 4: U = u1t.T@h + u1b.T@agg ; G = gelu(U + ub1)
                for dt in range(DT):
                    for (c0, cn) in CHUNKS:
                        pu = ps.tile([P, 512], f32, tag="pmm")
                        for kt in range(DT):
                            nc.tensor.matmul(
                                pu[:, :cn],
                                u1t[:, kt * D + dt * P: kt * D + dt * P + P],
                                hsl(hin, kt, c0, cn),
                                start=(kt == 0), stop=False,
                            )
                        for kt in range(DT):
                            nc.tensor.matmul(
                                pu[:, :cn],
                                u1b[:, kt * D + dt * P: kt * D + dt * P + P],
                                agg[:, kt * T + c0: kt * T + c0 + cn],
                                start=False, stop=(kt == DT - 1),
                            )
                        nc.scalar.activation(G[:, dt * T + c0: dt * T + c0 + cn],
                                             pu[:, :cn], AF.Gelu,
                                             bias=ub1[:, dt: dt + 1])

                # ---- stage 5: V = u2.T@G ; new_h = h + V + ub2 ; LN stats
                for dt in range(DT):
                    for ci, (c0, cn) in enumerate(CHUNKS):
                        pv = ps.tile([P, 512], f32, tag="pmm")
                        for kt in range(DT):
                            nc.tensor.matmul(
                                pv[:, :cn],
                                u2[:, kt * D + dt * P: kt * D + dt * P + P],
                                G[:, kt * T + c0: kt * T + c0 + cn],
                                start=(kt == 0), stop=False,
                            )
                        # residual: += I.T @ h  (identity matmul)
                        nc.tensor.matmul(
                            pv[:, :cn], iden[:], hsl(hin, dt, c0, cn),
                            start=False, stop=True,
                        )
                        # new_h (pre-LN) with ub2 bias; reuse hout as storage
                        nc.scalar.activation(hsl(hout, dt, c0, cn), pv[:, :cn],
                                             AF.Identity, bias=ub2[:, dt: dt + 1])
                        # x^2 for variance (bias applies before Square)
                        nc.scalar.activation(x2[:, dt * T + c0: dt * T + c0 + cn],
                                             pv[:, :cn], AF.Square,
                                             bias=ub2[:, dt: dt + 1])

                # ---- stage 6: LN over D (partition axis) via ones-matmuls
                for (c0, cn) in CHUNKS:
                    pr0 = psr.tile([1, 512], f32, tag="prow")
                    pr1 = psr.tile([1, 512], f32, tag="prow")
                    for kt in range(DT):
                        nc.tensor.matmul(pr0[:, :cn], ones_col[:],
                                         hsl(hout, kt, c0, cn),
                                         start=(kt == 0), stop=(kt == DT - 1))
                    for kt in range(DT):
                        nc.tensor.matmul(pr1[:, :cn], ones_col[:],
                                         x2[:, kt * T + c0: kt * T + c0 + cn],
                                         start=(kt == 0), stop=(kt == DT - 1))
                    nmu = rowAB[:, c0: c0 + cn]
                    rst = rowAB[:, T + c0: T + c0 + cn]
                    t0 = rowCD[:, c0: c0 + cn]
                    t1 = rowCD[:, T + c0: T + c0 + cn]
                    nc.vector.tensor_scalar_mul(nmu, pr0[:, :cn], -1.0 / D)
                    nc.vector.tensor_scalar_mul(t0, pr1[:, :cn], 1.0 / D)
                    # var = E[x^2] - mu^2
                    nc.vector.tensor_tensor(t1, nmu, nmu, ALU.mult)
                    nc.vector.tensor_tensor(t0, t0, t1, ALU.subtract)
                    nc.scalar.activation(t0, t0, AF.Sqrt, bias=EPS)
                    nc.vector.reciprocal(rst, t0)
                    # c0row = -mu * rstd  (in place)
                    nc.vector.tensor_tensor(nmu, nmu, rst, ALU.mult)
                    # broadcast across partitions via K=1 matmul
                    pb0 = ps.tile([P, 512], f32, tag="pmm")
                    pb1 = ps.tile([P, 512], f32, tag="pmm")
                    nc.tensor.matmul(pb0[:, :cn], ones_row[:], nmu, start=True, stop=True)
                    nc.tensor.matmul(pb1[:, :cn], ones_row[:], rst, start=True, stop=True)
                    for dt in range(DT):
                        xs = xn[:, dt * T + c0: dt * T + c0 + cn]
                        nc.vector.tensor_tensor(xs, hsl(hout, dt, c0, cn),
                                                pb1[:, :cn], ALU.mult)
                        nc.vector.tensor_tensor(xs, xs, pb0[:, :cn], ALU.add)
                        # h = xnorm * g + b (rounds to f32r on write)
                        nc.scalar.activation(hsl(hout, dt, c0, cn), xs,
                                             AF.Identity,
                                             scale=lng[:, dt: dt + 1],
                                             bias=lnb[:, dt: dt + 1])

                # zero the pad margin for sequence-start cores (identity on
                # halo cores); not needed after the final round
                if r < N_ROUNDS - 1:
                    for dt in range(DT):
                        nc.vector.tensor_tensor(
                            hsl(hout, dt, 0, HALO), hsl(hout, dt, 0, HALO),
                            hmask[:], ALU.mult)

            hfin = hbufs[N_ROUNDS % 2]
            for dt in range(DT):
                nc.sync.dma_start(
                    out=out_d[dt],
                    in_=hsl(hfin, dt, HALO, NLOC).bitcast(f32))

    nc.finalize()
    return nc


_NC_CACHE = {}


def _get_nc():
    if "nc" not in _NC_CACHE:
        _NC_CACHE["nc"] = build_nc()
    return _NC_CACHE["nc"]


def _prep_inputs(chunk_summaries, msg_w1, msg_b1, msg_w2, msg_b2,
                 upd_w1, upd_b1, upd_w2, upd_b2, ln_g, ln_b):
    h = np.asarray(chunk_summaries, np.float32)          # (B, N, D)
    w1 = np.asarray(msg_w1, np.float32)                  # (2D, D)
    w2 = np.asarray(msg_w2, np.float32)                  # (D, D)
    u1 = np.asarray(upd_w1, np.float32)
    u2 = np.asarray(upd_w2, np.float32)

    def pack_w(w, dt_np):
        # (D_in, D_out) -> (DT, P, D) K-tiled lhsT layout
        return np.ascontiguousarray(
            w.reshape(DT, P, D).astype(dt_np))

    def pack_b2(b):
        return np.ascontiguousarray(np.asarray(b, np.float32).reshape(DT, P).T)

    common = {
        "w1t": pack_w(w1[:D], np.float32),
        "w1b": pack_w(w1[D:], np.float32),
        "u1t": pack_w(u1[:D], np.float32),
        "u1b": pack_w(u1[D:], ml_dtypes.bfloat16),
        "w2s": pack_w(w2 / 9.0, ml_dtypes.bfloat16),
        "u2": pack_w(u2, ml_dtypes.bfloat16),
        "b1": pack_b2(msg_b1),
        "b2": pack_b2(msg_b2),
        "ub1": pack_b2(upd_b1),
        "ub2": pack_b2(upd_b2),
        "lng": pack_b2(ln_g),
        "lnb": pack_b2(ln_b),
        "iden": np.eye(P, dtype=np.float32),
    }

    # edge constants
    i8 = np.arange(W, dtype=np.float32)
    ea_edge = np.broadcast_to((W - i8), (P, W)).astype(ml_dtypes.bfloat16)
    es_edge = np.broadcast_to((9.0 / (i8 + 1.0)), (P, W)).astype(ml_dtypes.bfloat16)
    ea_mid = np.zeros((P, W), ml_dtypes.bfloat16)
    es_mid = np.ones((P, W), ml_dtypes.bfloat16)
    hm_edge = np.zeros((P, HALO), np.float32)
    hm_mid = np.ones((P, HALO), np.float32)

    in_maps = []
    for core in range(NCORES):
        b = core // 4
        q = core % 4
        n0 = q * NLOC
        if q == 0:
            loc = np.zeros((T, D), np.float32)
            loc[HALO:] = h[b, :NLOC]
            ea, es, hm = ea_edge, es_edge, hm_edge
        else:
            loc = h[b, n0 - HALO: n0 + NLOC]
            ea, es, hm = ea_mid, es_mid, hm_mid
        # (T, D) -> (DT, P, T)
        hloc = np.ascontiguousarray(loc.T.reshape(DT, P, T))
        m = dict(common)
        m["h_in"] = hloc
        m["edge_a"] = ea
        m["edge_s"] = es
        m["hmask"] = hm
        in_maps.append(m)
    return in_maps


def kernel(**inputs) -> np.ndarray:
    nc = _get_nc()
    in_maps = _prep_inputs(**inputs)
    res = run_bass_kernel_spmd(nc, in_maps, list(range(NCORES)))
    out = np.empty((B, N, D), np.float32)
    for core in range(NCORES):
        b = core // 4
        q = core % 4
        o = res.results[core]["out"]          # (DT, P, NLOC)
        out[b, q * NLOC:(q + 1) * NLOC] = o.reshape(D, NLOC).T
    return out


# revision 9
# speedup vs baseline: 1.1505x; 1.1505x over previous
"""Trainium2 Bass kernel for ChunkMessagePassing (gnn_message_passing).

Problem: B=2, N=4096, D=512, 3 rounds of causal windowed (W=8) message
passing. Per round:
    A = h @ w1_top + b1 ; Bv = h @ w1_bot          (first MLP layer, factored)
    S[i] = sum_{k=0..8, valid} gelu(A[i] + Bv[i-k])
    agg[i] = (S[i]/cnt[i]) @ w2 + b2               (sum commutes with linear)
    new_h = h + MLP_upd([h, agg]) ; h = LN(new_h)

Sharding: 8 cores = B(2) x N-quarters(4). Each core gets 1024 tokens plus a
24-token left halo (3 rounds x window 8), computed redundantly. Zero
cross-core communication. Cores at a sequence start get a zero pad instead
of a halo plus a data-driven edge fixup (invalid window taps excluded, window
count < 9) so all 8 cores run one SPMD program.

Layout: D on partitions (4 tiles of 128), tokens on the free axis.
Matmuls in fp32r (1 cyc/row, ~1e-4 rel err). Window stage in bf16 for DVE
2x mode; shifted reads stay 4B-aligned via an extra odd-shifted Bv copy.
LN stats via ones-matmul over partitions, broadcast back via K=1 matmul.
"""

import numpy as np
import ml_dtypes

import concourse.bacc as bacc
import concourse.mybir as mybir
from concourse.tile import TileContext
from concourse.bass_utils import run_bass_kernel_spmd

f32 = mybir.dt.float32
f32r = mybir.dt.float32r
bf16 = mybir.dt.bfloat16
AF = mybir.ActivationFunctionType
ALU = mybir.AluOpType

B, N, D = 2, 4096, 512
N_ROUNDS = 3
W = 8
W9 = W + 1
NCORES = 8
NLOC = N // 4            # tokens owned per core
HALO = N_ROUNDS * W      # 24
T = NLOC + HALO          # 1048 local tokens incl. halo
DT = 4                   # number of 128-partition d tiles
P = 128
MARG = 8                 # zero margin on the left of Bv buffers
CHUNKS = [(0, 352), (352, 352), (704, 344)]   # psum-stage chunks of T
HC = T // 2              # window-stage half chunk (524)
EPS = 1e-5


def build_nc():
    nc = bacc.Bacc("TRN2")

    # ---- DRAM I/O (per-core data supplied via in_maps) ----
    h_in = nc.dram_tensor("h_in", [DT, P, T], f32r, kind="ExternalInput")
    w1t_d = nc.dram_tensor("w1t", [DT, P, D], f32r, kind="ExternalInput")
    w1b_d = nc.dram_tensor("w1b", [DT, P, D], f32r, kind="ExternalInput")
    u1t_d = nc.dram_tensor("u1t", [DT, P, D], f32r, kind="ExternalInput")
    u1b_d = nc.dram_tensor("u1b", [DT, P, D], bf16, kind="ExternalInput")
    w2s_d = nc.dram_tensor("w2s", [DT, P, D], bf16, kind="ExternalInput")
    u2_d = nc.dram_tensor("u2", [DT, P, D], bf16, kind="ExternalInput")
    # biases packed (P, DT) column dt; ln gamma/beta too
    b1_d = nc.dram_tensor("b1", [P, DT], f32, kind="ExternalInput")
    b2_d = nc.dram_tensor("b2", [P, DT], f32, kind="ExternalInput")
    ub1_d = nc.dram_tensor("ub1", [P, DT], f32, kind="ExternalInput")
    ub2_d = nc.dram_tensor("ub2", [P, DT], f32, kind="ExternalInput")
    lng_d = nc.dram_tensor("lng", [P, DT], f32, kind="ExternalInput")
    lnb_d = nc.dram_tensor("lnb", [P, DT], f32, kind="ExternalInput")
    iden_d = nc.dram_tensor("iden", [P, P], f32r, kind="ExternalInput")
    # per-core edge constants
    ea_d = nc.dram_tensor("edge_a", [P, W], bf16, kind="ExternalInput")
    es_d = nc.dram_tensor("edge_s", [P, W], bf16, kind="ExternalInput")
    hm_d = nc.dram_tensor("hmask", [P, HALO], f32, kind="ExternalInput")
    out_d = nc.dram_tensor("out", [DT, P, NLOC], f32, kind="ExternalOutput")

    with nc.allow_low_precision("bf16/f32r compute validated against reference"), \
            TileContext(nc) as tc:
        with (
            tc.tile_pool(name="const", bufs=1) as cp,
            tc.tile_pool(name="acts", bufs=1) as ap,
            tc.tile_pool(name="wsc", bufs=2) as wp,
            tc.tile_pool(name="ps", bufs=6, space="PSUM") as ps,
            tc.tile_pool(name="psr", bufs=2, space="PSUM") as psr,
        ):
            # ---- constants into SBUF ----
            w1t = cp.tile([P, DT * D], f32r, tag="w1t")
            w1b = cp.tile([P, DT * D], f32r, tag="w1b")
            u1t = cp.tile([P, DT * D], f32r, tag="u1t")
            u1b = cp.tile([P, DT * D], bf16, tag="u1b")
            w2s = cp.tile([P, DT * D], bf16, tag="w2s")
            u2 = cp.tile([P, DT * D], bf16, tag="u2")
            iden = cp.tile([P, P], f32r, tag="iden")
            for t_sb, t_d in ((w1t, w1t_d), (w1b, w1b_d), (u1t, u1t_d),
                              (u1b, u1b_d), (w2s, w2s_d), (u2, u2_d)):
                for dt in range(DT):
                    nc.sync.dma_start(out=t_sb[:, dt * D:(dt + 1) * D], in_=t_d[dt])
            nc.sync.dma_start(out=iden[:], in_=iden_d[:])
            iden_b = cp.tile([P, P], bf16, tag="iden_b")
            nc.vector.tensor_copy(iden_b[:], iden[:].bitcast(f32))
            b1 = cp.tile([P, DT], f32, tag="b1")
            b2 = cp.tile([P, DT], f32, tag="b2")
            ub1 = cp.tile([P, DT], f32, tag="ub1")
            ub2 = cp.tile([P, DT], f32, tag="ub2")
            lng = cp.tile([P, DT], f32, tag="lng")
            lnb = cp.tile([P, DT], f32, tag="lnb")
            edge_a = cp.tile([P, W], bf16, tag="edge_a")
            edge_s = cp.tile([P, W], bf16, tag="edge_s")
            hmask = cp.tile([P, HALO], f32, tag="hmask")
            for t_sb, t_d in ((b1, b1_d), (b2, b2_d), (ub1, ub1_d), (ub2, ub2_d),
                              (lng, lng_d), (lnb, lnb_d), (edge_a, ea_d),
                              (edge_s, es_d), (hmask, hm_d)):
                nc.sync.dma_start(out=t_sb[:], in_=t_d[:])

            ones_col = cp.tile([P, 1], f32r, tag="ones_col")   # stats lhsT (K=128, M=1)
            ones_row = cp.tile([1, P], f32r, tag="ones_row")   # bcast lhsT (K=1, M=128)
            ones_f = cp.tile([P, 1], f32, tag="ones_f")
            nc.vector.memset(ones_f[:], 1.0)
            nc.vector.tensor_copy(ones_col[:], ones_f[:])
            nc.vector.tensor_copy(ones_row[:], ones_f[:1, :].to_broadcast([1, P]))
            # const APs for float biases in activation() (0.0 default, eps)
            czero = cp.tile([P, 1], f32, tag="czero")
            ceps = cp.tile([P, 1], f32, tag="ceps")
            nc.vector.memset(czero[:], 0.0)
            nc.vector.memset(ceps[:], EPS)
            nc.const_aps.aps[(f32, 0.0)] = czero[:]
            nc.const_aps.aps[(f32, EPS)] = ceps[:]

            # ---- activations (persistent, reused across rounds) ----
            h0 = ap.tile([P, DT * T], f32r, tag="h0")
            h1 = ap.tile([P, DT * T], f32r, tag="h1")
            A = ap.tile([P, DT * T], bf16, tag="A")
            BVW = MARG + T + 2        # small tail pad, keeps strides even
            Bv = ap.tile([P, DT * BVW], bf16, tag="Bv")
            Bvo = ap.tile([P, DT * BVW], bf16, tag="Bvo")
            S = ap.tile([P, DT * T], bf16, tag="S")
            agg = ap.tile([P, DT * T], bf16, tag="agg")
            x2 = ap.tile([P, DT * T], f32r, tag="x2")
            rowAB = ap.tile([1, 2 * T], f32r, tag="rowAB")   # [-mu*rstd | rstd]
            rowCD = ap.tile([1, 2 * T], f32, tag="rowCD")    # scratch
            ga8 = ap.tile([P, W], bf16, tag="ga8")
            # xn aliases x2: x2[*,c] is dead (stats consumed) before xn[*,c]
            # is written; Tile serializes the WAR. G aliases A likewise.
            xn = x2
            G = A

            # zero Bv margins once (per-dt row of MARG cols at block start)
            for dt in range(DT):
                nc.vector.memset(Bv[:, dt * BVW: dt * BVW + MARG], 0.0)
                nc.vector.memset(Bvo[:, dt * BVW: dt * BVW + MARG + 1], 0.0)

            # round-1 input: DMA straight into h0 (f32r dram -> f32r sbuf)
            for dt in range(DT):
                nc.sync.dma_start(out=h0[:, dt * T:(dt + 1) * T], in_=h_in[dt])

            def hsl(h, dt, c0, n):
                return h[:, dt * T + c0: dt * T + c0 + n]

            hbufs = [h0, h1]
            for r in range(N_ROUNDS):
                hin = hbufs[r % 2]
                hout = hbufs[(r + 1) % 2]

                # ---- stage 1: A = w1t.T@h (+b1 later via ACT bias), Bv = w1b.T@h
                for dt in range(DT):
                    for (c0, cn) in CHUNKS:
                        pa = ps.tile([P, 512], f32, tag="pmm")
                        for kt in range(DT):
                            nc.tensor.matmul(
                                pa[:, :cn],
                                w1t[:, kt * D + dt * P: kt * D + dt * P + P],
                                hsl(hin, kt, c0, cn),
                                start=(kt == 0), stop=(kt == DT - 1),
                            )
                        nc.scalar.activation(A[:, dt * T + c0: dt * T + c0 + cn],
                                             pa[:, :cn], AF.Copy)
                        pb = ps.tile([P, 512], f32, tag="pmm")
                        for kt in range(DT):
                            nc.tensor.matmul(
                                pb[:, :cn],
                                w1b[:, kt * D + dt * P: kt * D + dt * P + P],
                                hsl(hin, kt, c0, cn),
                                start=(kt == 0), stop=(kt == DT - 1),
                            )
                        base = dt * BVW + MARG + c0
                        nc.scalar.activation(Bv[:, base: base + cn], pb[:, :cn], AF.Copy)
                        nc.vector.tensor_copy(Bvo[:, base + 1: base + 1 + cn],
                                              Bv[:, base: base + cn])

                # ---- stage 2: windowed gelu-sum -> S
                # 9 shifted adds (DVE bf16 2x), one big gelu (ACT), then the
                # 9-block sum as identity-matmul psum accumulation - keeps PE
                # warm through the window stage instead of idling/cooling.
                for dt in range(DT):
                    for (c0, cn) in CHUNKS:
                        tmp = wp.tile([P, W9 * 352], bf16, tag="tmp")
                        g = wp.tile([P, W9 * 352], bf16, tag="g")
                        a_sl = A[:, dt * T + c0: dt * T + c0 + cn]
                        for k in range(W9):
                            # Bv[i-k]: even k from Bv (base MARG-k), odd k from
                            # Bvo (stored shifted +1; base MARG+1-k) - keeps
                            # 4B alignment for bf16 2x mode
                            if k % 2 == 0:
                                src = Bv[:, dt * BVW + MARG - k + c0:
                                         dt * BVW + MARG - k + c0 + cn]
                            else:
                                src = Bvo[:, dt * BVW + MARG + 1 - k + c0:
                                          dt * BVW + MARG + 1 - k + c0 + cn]
                            nc.vector.tensor_tensor(
                                tmp[:, k * cn:(k + 1) * cn], a_sl, src, ALU.add)
                        nc.scalar.activation(g[:, : W9 * cn], tmp[:, : W9 * cn],
                                             AF.Gelu, bias=b1[:, dt: dt + 1])
                        pS = ps.tile([P, 512], f32, tag="pmm")
                        for k in range(W9):
                            nc.tensor.matmul(pS[:, :cn], iden_b[:],
                                             g[:, k * cn:(k + 1) * cn],
                                             start=(k == 0), stop=(k == W9 - 1))
                        nc.scalar.activation(S[:, dt * T + c0: dt * T + c0 + cn],
                                             pS[:, :cn], AF.Copy)

                # ---- edge fixup (no-op on non-sequence-start cores) ----
                # S[i] := (S[i] - (W-i)*gelu(A[i]+b1)) * 9/(i+1) on local
                # columns HALO..HALO+8 (global tokens 0..7); edge_a/edge_s are
                # zeros/ones elsewhere by construction.
                for dt in range(DT):
                    sle = S[:, dt * T + HALO: dt * T + HALO + W]
                    nc.scalar.activation(ga8[:], A[:, dt * T + HALO: dt * T + HALO + W],
                                         AF.Gelu, bias=b1[:, dt: dt + 1])
                    nc.vector.tensor_tensor(ga8[:], ga8[:], edge_a[:], ALU.mult)
                    nc.vector.tensor_tensor(sle, sle, ga8[:], ALU.subtract)
                    nc.vector.tensor_tensor(sle, sle, edge_s[:], ALU.mult)

                # ---- stage 3: agg = S @ w2s + b2 (w2s pre-scaled by 1/9)
                for dt in range(DT):
                    for (c0, cn) in CHUNKS:
                        pg = ps.tile([P, 512], f32, tag="pmm")
                        for kt in range(DT):
                            nc.tensor.matmul(
                                pg[:, :cn],
                                w2s[:, kt * D + dt * P: kt * D + dt * P + P],
                                S[:, kt * T + c0: kt * T + c0 + cn],
                                start=(kt == 0), stop=(kt == DT - 1),
                            )
                        nc.scalar.activation(agg[:, dt * T + c0: dt * T + c0 + cn],
                                             pg[:, :cn], AF.Identity,
                                             bias=b2[:, dt: dt + 1])

                # ---- stage 4: U = u1t.T@h + u1b.T@agg ; G = gelu(U + ub1)
                for dt in range(DT):
                    for (c0, cn) in CHUNKS:
                        pu = ps.tile([P, 512], f32, tag="pmm")
                        for kt in range(DT):
                            nc.tensor.matmul(
                                pu[:, :cn],
                                u1t[:, kt * D + dt * P: kt * D + dt * P + P],
                                hsl(hin, kt, c0, cn),
                                start=(kt == 0), stop=False,
                            )
                        for kt in range(DT):
                            nc.tensor.matmul(
                                pu[:, :cn],
                                u1b[:, kt * D + dt * P: kt * D + dt * P + P],
                                agg[:, kt * T + c0: kt * T + c0 + cn],
                                start=False, stop=(kt == DT - 1),
                            )
                        nc.scalar.activation(G[:, dt * T + c0: dt * T + c0 + cn],
                                             pu[:, :cn], AF.Gelu,
                                             bias=ub1[:, dt: dt + 1])

                # ---- stage 5: V = u2.T@G ; new_h = h + V + ub2 ; LN stats
                for dt in range(DT):
                    for ci, (c0, cn) in enumerate(CHUNKS):
                        pv = ps.tile([P, 512], f32, tag="pmm")
                        for kt in range(DT):
                            nc.tensor.matmul(
                                pv[:, :cn],
                                u2[:, kt * D + dt * P: kt * D + dt * P + P],
                                G[:, kt * T + c0: kt * T + c0 + cn],
                                start=(kt == 0), stop=False,
                            )
                        # residual: += I.T @ h  (identity matmul)
                        nc.tensor.matmul(
                            pv[:, :cn], iden[:], hsl(hin, dt, c0, cn),
                            start=False, stop=True,
                        )
                        # new_h (pre-LN) with ub2 bias; reuse hout as storage
                        nc.scalar.activation(hsl(hout, dt, c0, cn), pv[:, :cn],
                                             AF.Identity, bias=ub2[:, dt: dt + 1])
                        # x^2 for variance (DVE, frees ACT)
                        nc.vector.tensor_tensor(
                            x2[:, dt * T + c0: dt * T + c0 + cn],
                            hsl(hout, dt, c0, cn), hsl(hout, dt, c0, cn),
                            ALU.mult)

                # ---- stage 6: LN over D (partition axis) via ones-matmuls
                for (c0, cn) in CHUNKS:
                    pr0 = psr.tile([1, 512], f32, tag="prow")
                    pr1 = psr.tile([1, 512], f32, tag="prow")
                    for kt in range(DT):
                        nc.tensor.matmul(pr0[:, :cn], ones_col[:],
                                         hsl(hout, kt, c0, cn),
                                         start=(kt == 0), stop=(kt == DT - 1))
                    for kt in range(DT):
                        nc.tensor.matmul(pr1[:, :cn], ones_col[:],
                                         x2[:, kt * T + c0: kt * T + c0 + cn],
                                         start=(kt == 0), stop=(kt == DT - 1))
                    nmu = rowAB[:, c0: c0 + cn]
                    rst = rowAB[:, T + c0: T + c0 + cn]
                    t0 = rowCD[:, c0: c0 + cn]
                    t1 = rowCD[:, T + c0: T + c0 + cn]
                    nc.vector.tensor_scalar_mul(nmu, pr0[:, :cn], -1.0 / D)
                    nc.vector.tensor_scalar_mul(t0, pr1[:, :cn], 1.0 / D)
                    # var = E[x^2] - mu^2
                    nc.vector.tensor_tensor(t1, nmu, nmu, ALU.mult)
                    nc.vector.tensor_tensor(t0, t0, t1, ALU.subtract)
                    nc.scalar.activation(t1, t0, AF.Ln, bias=EPS)
                    nc.scalar.activation(rst, t1, AF.Exp, scale=-0.5)
                    # c0row = -mu * rstd  (in place)
                    nc.vector.tensor_tensor(nmu, nmu, rst, ALU.mult)
                    # broadcast across partitions via K=1 matmul
                    pb0 = ps.tile([P, 512], f32, tag="pmm")
                    pb1 = ps.tile([P, 512], f32, tag="pmm")
                    nc.tensor.matmul(pb0[:, :cn], ones_row[:], nmu, start=True, stop=True)
                    nc.tensor.matmul(pb1[:, :cn], ones_row[:], rst, start=True, stop=True)
                    for dt in range(DT):
                        xs = xn[:, dt * T + c0: dt * T + c0 + cn]
                        nc.vector.tensor_tensor(xs, hsl(hout, dt, c0, cn),
                                                pb1[:, :cn], ALU.mult)
                        nc.vector.tensor_tensor(xs, xs, pb0[:, :cn], ALU.add)
                        # h = xnorm * g + b (rounds to f32r on write)
                        nc.scalar.activation(hsl(hout, dt, c0, cn), xs,
                                             AF.Identity,
                                             scale=lng[:, dt: dt + 1],
                                             bias=lnb[:, dt: dt + 1])

                # zero the pad margin for sequence-start cores (identity on
                # halo cores); not needed after the final round
                if r < N_ROUNDS - 1:
                    for dt in range(DT):
                        nc.vector.tensor_tensor(
                            hsl(hout, dt, 0, HALO), hsl(hout, dt, 0, HALO),
                            hmask[:], ALU.mult)

            hfin = hbufs[N_ROUNDS % 2]
            for dt in range(DT):
                nc.sync.dma_start(
                    out=out_d[dt],
                    in_=hsl(hfin, dt, HALO, NLOC).bitcast(f32))

    nc.finalize()
    return nc


_NC_CACHE = {}


def _get_nc():
    if "nc" not in _NC_CACHE:
        _NC_CACHE["nc"] = build_nc()
    return _NC_CACHE["nc"]


def _prep_inputs(chunk_summaries, msg_w1, msg_b1, msg_w2, msg_b2,
                 upd_w1, upd_b1, upd_w2, upd_b2, ln_g, ln_b):
    h = np.asarray(chunk_summaries, np.float32)          # (B, N, D)
    w1 = np.asarray(msg_w1, np.float32)                  # (2D, D)
    w2 = np.asarray(msg_w2, np.float32)                  # (D, D)
    u1 = np.asarray(upd_w1, np.float32)
    u2 = np.asarray(upd_w2, np.float32)

    def pack_w(w, dt_np):
        # (D_in, D_out) -> (DT, P, D) K-tiled lhsT layout
        return np.ascontiguousarray(
            w.reshape(DT, P, D).astype(dt_np))

    def pack_b2(b):
        return np.ascontiguousarray(np.asarray(b, np.float32).reshape(DT, P).T)

    common = {
        "w1t": pack_w(w1[:D], np.float32),
        "w1b": pack_w(w1[D:], np.float32),
        "u1t": pack_w(u1[:D], np.float32),
        "u1b": pack_w(u1[D:], ml_dtypes.bfloat16),
        "w2s": pack_w(w2 / 9.0, ml_dtypes.bfloat16),
        "u2": pack_w(u2, ml_dtypes.bfloat16),
        "b1": pack_b2(msg_b1),
        "b2": pack_b2(msg_b2),
        "ub1": pack_b2(upd_b1),
        "ub2": pack_b2(upd_b2),
        "lng": pack_b2(ln_g),
        "lnb": pack_b2(ln_b),
        "iden": np.eye(P, dtype=np.float32),
    }

    # edge constants
    i8 = np.arange(W, dtype=np.float32)
    ea_edge = np.broadcast_to((W - i8), (P, W)).astype(ml_dtypes.bfloat16)
    es_edge = np.broadcast_to((9.0 / (i8 + 1.0)), (P, W)).astype(ml_dtypes.bfloat16)
    ea_mid = np.zeros((P, W), ml_dtypes.bfloat16)
    es_mid = np.ones((P, W), ml_dtypes.bfloat16)
    hm_edge = np.zeros((P, HALO), np.float32)
    hm_mid = np.ones((P, HALO), np.float32)

    in_maps = []
    for core in range(NCORES):
        b = core // 4
        q = core % 4
        n0 = q * NLOC
        if q == 0:
            loc = np.zeros((T, D), np.float32)
            loc[HALO:] = h[b, :NLOC]
            ea, es, hm = ea_edge, es_edge, hm_edge
        else:
            loc = h[b, n0 - HALO: n0 + NLOC]
            ea, es, hm = ea_mid, es_mid, hm_mid
        # (T, D) -> (DT, P, T)
        hloc = np.ascontiguousarray(loc.T.reshape(DT, P, T))
        m = dict(common)
        m["h_in"] = hloc
        m["edge_a"] = ea
        m["edge_s"] = es
        m["hmask"] = hm
        in_maps.append(m)
    return in_maps


def kernel(**inputs) -> np.ndarray:
    nc = _get_nc()
    in_maps = _prep_inputs(**inputs)
    res = run_bass_kernel_spmd(nc, in_maps, list(range(NCORES)))
    out = np.empty((B, N, D), np.float32)
    for core in range(NCORES):
        b = core // 4
        q = core % 4
        o = res.results[core]["out"]          # (DT, P, NLOC)
        out[b, q * NLOC:(q + 1) * NLOC] = o.reshape(D, NLOC).T
    return out
